# revision 26
# baseline (speedup 1.0000x reference)
"""MoE PHM-MLP kernel for 8 Trainium2 NeuronCores (expert parallelism).

Strategy:
  - The PHM layer (sum_i kron(A_i, S_i)) is folded on the host into dense
    per-expert weights:  W_fc[e]  : [HID, D],  W_proj[e] : [D, HID].
  - Routing (router matmul + softmax + argmax + aux loss) runs on the host:
    the gather/dispatch indices are needed before the device launch anyway.
  - Tokens are gathered per expert (capacity = max count rounded to 64) and
    each of the 8 cores runs one expert's 2-layer MLP in bf16:
        yT = W2^T @ square(leaky_relu(W1^T @ xgT))
    with xgT: [D, C] so the contraction dim always sits on SBUF partitions.
  - Host scatters rows back; no collectives are needed.
"""

import sys

import numpy as np

for _p in ("/opt/trn_rl_repo", "/root/.axon_site/_ro/trn_rl_repo"):
    if _p not in sys.path:
        sys.path.append(_p)

import ml_dtypes

BF16 = ml_dtypes.bfloat16

P = 128
B, S, D = 2, 2048, 1024
E, N, HID = 8, 4, 4096
T = B * S
N_CORES = 8

_program_cache: dict[int, object] = {}


def _build_program(C: int):
    """Bass program for one expert-core: [D,C] tokens -> [D,C] output.

    Token chunks (512 + remainder) are interleaved inside the k loop so
    consecutive matmuls share the stationary weight tile.  xg/w1 DMAs are
    emitted first (per-queue FIFO gives them the early HBM bandwidth) and
    layer-2 weights for k16..31 are DMA'd into w1's SBUF slots once layer 1
    has released them, which keeps total SBUF under budget.
    """
    import concourse.bacc as bacc
    import concourse.mybir as mybir
    import concourse.tile as tile

    nc = bacc.Bacc()
    xgT = nc.declare_dram_parameter("xgT", [D, C], mybir.dt.bfloat16, isOutput=False)
    w1 = nc.declare_dram_parameter("w1", [D, HID], mybir.dt.bfloat16, isOutput=False)
    w2 = nc.declare_dram_parameter("w2", [HID, D], mybir.dt.bfloat16, isOutput=False)
    out = nc.declare_dram_parameter("out", [D, C], mybir.dt.float32, isOutput=True)

    K1 = D // P    # 8  k-tiles for layer 1 (contraction over D)
    HO = HID // P  # 32 hid tiles (layer-1 outputs / layer-2 contraction)
    DO = D // P    # 8  output row tiles for layer 2

    chunks = []
    c0 = 0
    while c0 < C:
        f = min(512, C - c0)
        chunks.append((c0, f))
        c0 += f
    NCH = len(chunks)
    assert NCH <= 4, f"capacity {C} too large"

    AF = mybir.ActivationFunctionType
    BF = mybir.dt.bfloat16
    F32 = mybir.dt.float32

    with tile.TileContext(nc) as tc:
        with (
            tc.tile_pool(name="wpool", bufs=1) as wpool,
            tc.tile_pool(name="xpool", bufs=1) as xpool,
            tc.tile_pool(name="h2pool", bufs=HO) as h2pool,
            tc.tile_pool(name="actpool", bufs=3) as actpool,
            tc.tile_pool(name="ypool", bufs=3) as ypool,
            tc.tile_pool(name="pspool", bufs=2, space="PSUM") as pspool,
        ):
            # --- input DMAs, in arrival-priority order.  w1 is split into
            # hid-QUARTERS: the first 4 in-flight ho groups (0..7) need only
            # quarter 0 (2MB+xg), so the PE runs dense almost immediately;
            # later quarters stream in while earlier ho tiles compute. ---
            QW = HID // 4
            NQ = 4
            xg_t = []
            w1q_t = [[[None] * 2 for _ in range(K1)] for _ in range(2)]
            for ko in range(K1):
                tx = xpool.tile([P, C], BF, tag=f"xg{ko}", name=f"xg{ko}")
                nc.sync.dma_start(out=tx[:], in_=xgT[ko * P:(ko + 1) * P, :])
                xg_t.append(tx)
            for quarter in range(NQ):
                half, q = divmod(quarter, 2)
                for ko in range(K1):
                    tw = wpool.tile([P, QW], BF, tag=f"w1_{ko}_{half}_{q}",
                                    name=f"w1_{ko}_{half}_{q}")
                    nc.sync.dma_start(
                        out=tw[:],
                        in_=w1[ko * P:(ko + 1) * P,
                               quarter * QW:(quarter + 1) * QW])
                    w1q_t[half][ko][q] = tw

            def w2_slice(ko, do):
                """lhsT [128, 128] for layer-2 k-tile ko, output tile do."""
                if ko < 16:
                    return w2lo_t[ko][:, do * P:(do + 1) * P]
                return w2hi_t[ko - 16][:, do * P:(do + 1) * P]

            PS_BUFS = 8 // NCH

            def mk_psums(nm):
                ps = []
                for ci, (c0, f) in enumerate(chunks):
                    t = pspool.tile([P, 512], F32, tag=f"ps{ci}", bufs=PS_BUFS,
                                    name=f"{nm}{ci}")[:, :f]
                    ps.append(t)
                return ps

            # --- layer 1: full k groups, chunks interleaved per stationary ---
            h2_t = []
            for ho in range(HO):
                ps = mk_psums("p1")
                half, hh = divmod(ho, HO // 2)
                q, hq = divmod(hh, HO // 4)
                for ko in range(K1):
                    lhs = w1q_t[half][ko][q][:, hq * P:(hq + 1) * P]
                    for ci, (c0, f) in enumerate(chunks):
                        nc.tensor.matmul(ps[ci], lhsT=lhs,
                                         rhs=xg_t[ko][:, c0:c0 + f],
                                         start=(ko == 0), stop=(ko == K1 - 1))
                h2 = h2pool.tile([P, C], BF, tag="h2", name="h2")
                for ci, (c0, f) in enumerate(chunks):
                    tmp = actpool.tile([P, f], F32, tag=f"act{ci}",
                                       name=f"act{ci}")
                    # Prelu honors alpha on HW (parametric_relu is in every
                    # ACT table set); Lrelu silently degrades to plain relu.
                    nc.scalar.activation(out=tmp, in_=ps[ci], func=AF.Prelu, alpha=0.5)
                    nc.scalar.activation(out=h2[:, c0:c0 + f], in_=tmp, func=AF.Square)
                h2_t.append(h2)

            # --- layer-2 weights: emitted after layer 1 so per-queue FIFO
            # order gives the w1/xg DMAs the early HBM bandwidth ---
            w2lo_t = []
            for ko in range(16):
                t = wpool.tile([P, D], BF, tag=f"w2_{ko}", name=f"w2_{ko}")
                nc.sync.dma_start(out=t[:], in_=w2[ko * P:(ko + 1) * P, :])
                w2lo_t.append(t)
            # k16..31 reuse w1's half-0 quarter slots 1:1 (each [P, D],
            # free as soon as layer-1 ho 0..15 finish)
            w2hi_t = []
            for s in range(16):
                koq, q = divmod(s, 2)
                t = wpool.tile([P, QW], BF, tag=f"w1_{koq}_0_{q}", name=f"w2hi_{s}")
                nc.sync.dma_start(out=t[:], in_=w2[(16 + s) * P:(17 + s) * P, :])
                w2hi_t.append(t)

            # --- layer 2 ---
            for do in range(DO):
                ps = mk_psums("p2")
                for ko in range(HO):
                    for ci, (c0, f) in enumerate(chunks):
                        nc.tensor.matmul(ps[ci], lhsT=w2_slice(ko, do),
                                         rhs=h2_t[ko][:, c0:c0 + f],
                                         start=(ko == 0), stop=(ko == HO - 1))
                for ci, (c0, f) in enumerate(chunks):
                    yt = ypool.tile([P, f], F32, tag=f"yt{ci}",
                                    name=f"yt{ci}")
                    nc.vector.tensor_copy(out=yt, in_=ps[ci])
                    nc.sync.dma_start(out=out[do * P:(do + 1) * P, c0:c0 + f],
                                        in_=yt)
    nc.finalize()
    return nc


def _get_program(C: int):
    if C not in _program_cache:
        _program_cache[C] = _build_program(C)
    return _program_cache[C]


def kernel(x, w_router, A_fc, S_fc, A_proj, S_proj, _trace=False):
    from concourse.bass_utils import run_bass_kernel_spmd

    x = np.asarray(x, np.float32)
    w_router = np.asarray(w_router, np.float32)
    A_fc = np.asarray(A_fc, np.float32)
    S_fc = np.asarray(S_fc, np.float32)
    A_proj = np.asarray(A_proj, np.float32)
    S_proj = np.asarray(S_proj, np.float32)

    flat = x.reshape(T, D)

    # --- routing (host) ---
    logits = flat @ w_router.T
    m = logits.max(axis=-1, keepdims=True)
    p = np.exp(logits - m)
    probs = p / p.sum(axis=-1, keepdims=True)
    eidx = logits.argmax(axis=-1)
    counts = np.bincount(eidx, minlength=E)
    frac = counts.astype(np.float32) / np.float32(T)
    aux_loss = np.asarray((frac * probs.mean(axis=0)).sum() * E, dtype=np.float32)

    # --- fold PHM weights into dense matrices, pre-transposed ---
    # W_fc[e][(j,o),(k,l)] = sum_i A_fc[e,i,j,k] * S_fc[e,i,o,l]; w1 = W_fc[e].T
    w1_all = np.einsum("eijk,eiol->ekljo", A_fc, S_fc).reshape(E, D, HID)
    w2_all = np.einsum("eijk,eiol->ekljo", A_proj, S_proj).reshape(E, HID, D)
    w1_all = np.ascontiguousarray(w1_all).astype(BF16)
    w2_all = np.ascontiguousarray(w2_all).astype(BF16)

    # --- dispatch (host gather); capacity rounded to 64 tokens ---
    C = max(P, int(-(-int(counts.max()) // 64)) * 64)
    if C > 896:
        # Pathologically skewed routing (never happens for randn inputs):
        # fall back to exact numpy so correctness never depends on SBUF fit.
        out = np.zeros((T, D), np.float32)
        W_fc = np.einsum("eijk,eiol->ejokl", A_fc, S_fc).reshape(E, HID, D)
        W_proj = np.einsum("eijk,eiol->ejokl", A_proj, S_proj).reshape(E, D, HID)
        for e in range(E):
            sel = eidx == e
            if not sel.any():
                continue
            h = flat[sel] @ W_fc[e].T
            h = np.where(h >= 0, h, np.float32(0.5) * h)
            out[sel] = (h * h) @ W_proj[e].T
        return out.reshape(x.shape), aux_loss
    idx = [np.nonzero(eidx == e)[0] for e in range(E)]
    flat_bf = flat.astype(BF16)
    in_maps = []
    for e in range(E):
        xgT = np.zeros((D, C), BF16)
        xgT[:, :counts[e]] = flat_bf[idx[e]].T
        in_maps.append({"xgT": xgT, "w1": w1_all[e], "w2": w2_all[e]})

    nc = _get_program(C)
    kw = {"trace": True, "trace_cores": list(range(N_CORES))} if _trace else {}
    res = run_bass_kernel_spmd(nc, in_maps, list(range(N_CORES)), **kw)

    # --- scatter back (host) ---
    out_flat = np.empty((T, D), np.float32)
    for e in range(E):
        out_flat[idx[e]] = res.results[e]["out"][:, :counts[e]].T

    out = out_flat.reshape(x.shape)
    if _trace:
        return (out, aux_loss), res
    return out, aux_loss


# revision 27
# speedup vs baseline: 1.0435x; 1.0435x over previous
"""MoE PHM-MLP kernel for 8 Trainium2 NeuronCores (expert parallelism).

Strategy:
  - The PHM layer (sum_i kron(A_i, S_i)) is folded on the host into dense
    per-expert weights:  W_fc[e]  : [HID, D],  W_proj[e] : [D, HID].
  - Routing (router matmul + softmax + argmax + aux loss) runs on the host:
    the gather/dispatch indices are needed before the device launch anyway.
  - Tokens are gathered per expert (capacity = max count rounded to 64) and
    each of the 8 cores runs one expert's 2-layer MLP in bf16:
        yT = W2^T @ square(leaky_relu(W1^T @ xgT))
    with xgT: [D, C] so the contraction dim always sits on SBUF partitions.
  - Host scatters rows back; no collectives are needed.
"""

import sys

import numpy as np

for _p in ("/opt/trn_rl_repo", "/root/.axon_site/_ro/trn_rl_repo"):
    if _p not in sys.path:
        sys.path.append(_p)

import ml_dtypes

BF16 = ml_dtypes.bfloat16

P = 128
B, S, D = 2, 2048, 1024
E, N, HID = 8, 4, 4096
T = B * S
N_CORES = 8

_program_cache: dict[int, object] = {}


def _build_program(C: int):
    """Bass program for one expert-core: [D,C] tokens -> [D,C] output.

    Token chunks (512 + remainder) are interleaved inside the k loop so
    consecutive matmuls share the stationary weight tile.  xg/w1 DMAs are
    emitted first (per-queue FIFO gives them the early HBM bandwidth) and
    layer-2 weights for k16..31 are DMA'd into w1's SBUF slots once layer 1
    has released them, which keeps total SBUF under budget.
    """
    import concourse.bacc as bacc
    import concourse.mybir as mybir
    import concourse.tile as tile

    nc = bacc.Bacc()
    xgT = nc.declare_dram_parameter("xgT", [D, C], mybir.dt.bfloat16, isOutput=False)
    w1 = nc.declare_dram_parameter("w1", [D, HID], mybir.dt.bfloat16, isOutput=False)
    w2 = nc.declare_dram_parameter("w2", [HID, D], mybir.dt.bfloat16, isOutput=False)
    out = nc.declare_dram_parameter("out", [D, C], mybir.dt.float32, isOutput=True)

    K1 = D // P    # 8  k-tiles for layer 1 (contraction over D)
    HO = HID // P  # 32 hid tiles (layer-1 outputs / layer-2 contraction)
    DO = D // P    # 8  output row tiles for layer 2

    chunks = []
    c0 = 0
    while c0 < C:
        f = min(512, C - c0)
        chunks.append((c0, f))
        c0 += f
    NCH = len(chunks)
    assert NCH <= 4, f"capacity {C} too large"

    AF = mybir.ActivationFunctionType
    BF = mybir.dt.bfloat16
    F32 = mybir.dt.float32

    with tile.TileContext(nc) as tc:
        with (
            tc.tile_pool(name="wpool", bufs=1) as wpool,
            tc.tile_pool(name="xpool", bufs=1) as xpool,
            tc.tile_pool(name="h2pool", bufs=HO) as h2pool,
            tc.tile_pool(name="actpool", bufs=3) as actpool,
            tc.tile_pool(name="ypool", bufs=3) as ypool,
            tc.tile_pool(name="pspool", bufs=2, space="PSUM") as pspool,
        ):
            # --- input DMAs, in arrival-priority order.  w1 is split into
            # hid-QUARTERS: the first 4 in-flight ho groups (0..7) need only
            # quarter 0 (2MB+xg), so the PE runs dense almost immediately;
            # later quarters stream in while earlier ho tiles compute. ---
            QW = HID // 4
            NQ = 4
            xg_t = []
            w1q_t = [[[None] * 2 for _ in range(K1)] for _ in range(2)]
            for quarter in range(NQ):
                half, q = divmod(quarter, 2)
                for ko in range(K1):
                    if quarter == 0:
                        tx = xpool.tile([P, C], BF, tag=f"xg{ko}", name=f"xg{ko}")
                        nc.sync.dma_start(out=tx[:], in_=xgT[ko * P:(ko + 1) * P, :])
                        xg_t.append(tx)
                    tw = wpool.tile([P, QW], BF, tag=f"w1_{ko}_{half}_{q}",
                                    name=f"w1_{ko}_{half}_{q}")
                    nc.sync.dma_start(
                        out=tw[:],
                        in_=w1[ko * P:(ko + 1) * P,
                               quarter * QW:(quarter + 1) * QW])
                    w1q_t[half][ko][q] = tw

            def w2_slice(ko, do):
                """lhsT [128, 128] for layer-2 k-tile ko, output tile do."""
                if ko < 16:
                    return w2lo_t[ko][:, do * P:(do + 1) * P]
                return w2hi_t[ko - 16][:, do * P:(do + 1) * P]

            PS_BUFS = 8 // NCH

            def mk_psums(nm):
                ps = []
                for ci, (c0, f) in enumerate(chunks):
                    t = pspool.tile([P, 512], F32, tag=f"ps{ci}", bufs=PS_BUFS,
                                    name=f"{nm}{ci}")[:, :f]
                    ps.append(t)
                return ps

            # --- layer 1: full k groups, chunks interleaved per stationary ---
            h2_t = []
            for ho in range(HO):
                ps = mk_psums("p1")
                half, hh = divmod(ho, HO // 2)
                q, hq = divmod(hh, HO // 4)
                for ko in range(K1):
                    lhs = w1q_t[half][ko][q][:, hq * P:(hq + 1) * P]
                    for ci, (c0, f) in enumerate(chunks):
                        nc.tensor.matmul(ps[ci], lhsT=lhs,
                                         rhs=xg_t[ko][:, c0:c0 + f],
                                         start=(ko == 0), stop=(ko == K1 - 1))
                h2 = h2pool.tile([P, C], BF, tag="h2", name="h2")
                for ci, (c0, f) in enumerate(chunks):
                    tmp = actpool.tile([P, f], F32, tag=f"act{ci}",
                                       name=f"act{ci}")
                    # Prelu honors alpha on HW (parametric_relu is in every
                    # ACT table set); Lrelu silently degrades to plain relu.
                    nc.scalar.activation(out=tmp, in_=ps[ci], func=AF.Prelu, alpha=0.5)
                    nc.scalar.activation(out=h2[:, c0:c0 + f], in_=tmp, func=AF.Square)
                h2_t.append(h2)

            # --- layer-2 weights: emitted after layer 1 so per-queue FIFO
            # order gives the w1/xg DMAs the early HBM bandwidth ---
            w2lo_t = []
            for ko in range(16):
                t = wpool.tile([P, D], BF, tag=f"w2_{ko}", name=f"w2_{ko}")
                nc.sync.dma_start(out=t[:], in_=w2[ko * P:(ko + 1) * P, :])
                w2lo_t.append(t)
            # k16..31 reuse w1's half-0 quarter slots 1:1 (each [P, D],
            # free as soon as layer-1 ho 0..15 finish)
            w2hi_t = []
            for s in range(16):
                koq, q = divmod(s, 2)
                t = wpool.tile([P, QW], BF, tag=f"w1_{koq}_0_{q}", name=f"w2hi_{s}")
                nc.sync.dma_start(out=t[:], in_=w2[(16 + s) * P:(17 + s) * P, :])
                w2hi_t.append(t)

            # --- layer 2 ---
            for do in range(DO):
                ps = mk_psums("p2")
                for ko in range(HO):
                    for ci, (c0, f) in enumerate(chunks):
                        nc.tensor.matmul(ps[ci], lhsT=w2_slice(ko, do),
                                         rhs=h2_t[ko][:, c0:c0 + f],
                                         start=(ko == 0), stop=(ko == HO - 1))
                for ci, (c0, f) in enumerate(chunks):
                    yt = ypool.tile([P, f], F32, tag=f"yt{ci}",
                                    name=f"yt{ci}")
                    nc.vector.tensor_copy(out=yt, in_=ps[ci])
                    nc.sync.dma_start(out=out[do * P:(do + 1) * P, c0:c0 + f],
                                        in_=yt)
    nc.finalize()
    return nc


def _get_program(C: int):
    if C not in _program_cache:
        _program_cache[C] = _build_program(C)
    return _program_cache[C]


def kernel(x, w_router, A_fc, S_fc, A_proj, S_proj, _trace=False):
    from concourse.bass_utils import run_bass_kernel_spmd

    x = np.asarray(x, np.float32)
    w_router = np.asarray(w_router, np.float32)
    A_fc = np.asarray(A_fc, np.float32)
    S_fc = np.asarray(S_fc, np.float32)
    A_proj = np.asarray(A_proj, np.float32)
    S_proj = np.asarray(S_proj, np.float32)

    flat = x.reshape(T, D)

    # --- routing (host) ---
    logits = flat @ w_router.T
    m = logits.max(axis=-1, keepdims=True)
    p = np.exp(logits - m)
    probs = p / p.sum(axis=-1, keepdims=True)
    eidx = logits.argmax(axis=-1)
    counts = np.bincount(eidx, minlength=E)
    frac = counts.astype(np.float32) / np.float32(T)
    aux_loss = np.asarray((frac * probs.mean(axis=0)).sum() * E, dtype=np.float32)

    # --- fold PHM weights into dense matrices, pre-transposed ---
    # W_fc[e][(j,o),(k,l)] = sum_i A_fc[e,i,j,k] * S_fc[e,i,o,l]; w1 = W_fc[e].T
    w1_all = np.einsum("eijk,eiol->ekljo", A_fc, S_fc).reshape(E, D, HID)
    w2_all = np.einsum("eijk,eiol->ekljo", A_proj, S_proj).reshape(E, HID, D)
    w1_all = np.ascontiguousarray(w1_all).astype(BF16)
    w2_all = np.ascontiguousarray(w2_all).astype(BF16)

    # --- dispatch (host gather); capacity rounded to 64 tokens ---
    C = max(P, int(-(-int(counts.max()) // 64)) * 64)
    if C > 896:
        # Pathologically skewed routing (never happens for randn inputs):
        # fall back to exact numpy so correctness never depends on SBUF fit.
        out = np.zeros((T, D), np.float32)
        W_fc = np.einsum("eijk,eiol->ejokl", A_fc, S_fc).reshape(E, HID, D)
        W_proj = np.einsum("eijk,eiol->ejokl", A_proj, S_proj).reshape(E, D, HID)
        for e in range(E):
            sel = eidx == e
            if not sel.any():
                continue
            h = flat[sel] @ W_fc[e].T
            h = np.where(h >= 0, h, np.float32(0.5) * h)
            out[sel] = (h * h) @ W_proj[e].T
        return out.reshape(x.shape), aux_loss
    idx = [np.nonzero(eidx == e)[0] for e in range(E)]
    flat_bf = flat.astype(BF16)
    in_maps = []
    for e in range(E):
        xgT = np.zeros((D, C), BF16)
        xgT[:, :counts[e]] = flat_bf[idx[e]].T
        in_maps.append({"xgT": xgT, "w1": w1_all[e], "w2": w2_all[e]})

    nc = _get_program(C)
    kw = {"trace": True, "trace_cores": list(range(N_CORES))} if _trace else {}
    res = run_bass_kernel_spmd(nc, in_maps, list(range(N_CORES)), **kw)

    # --- scatter back (host) ---
    out_flat = np.empty((T, D), np.float32)
    for e in range(E):
        out_flat[idx[e]] = res.results[e]["out"][:, :counts[e]].T

    out = out_flat.reshape(x.shape)
    if _trace:
        return (out, aux_loss), res
    return out, aux_loss
